# revision 1
# baseline (speedup 1.0000x reference)
"""Trainium2 Bass kernel for nn_Attn_48206712930921.

softmax over s of energies[b,s] where energies[b,s] = outputs[b,s,:].v + c,
v = W^T @ weight_vec, c = weight_vec.b  (the [H,H] projection collapses to a
length-H dot product).  Rows s >= text_lens[b] softmax to exactly 0 (the
-1e10 fill underflows exp), so only the valid prefix of each sequence is
ever read: ~49.5% of the input.

Ragged schedule: each batch b occupies ceil(len_b/128) 128-row chunks;
whole batches are LPT-packed onto the 8 cores (near-perfect balance).  The
host packs each core's valid rows as bf16 in a [128, NCOL, H] layout so
every DMA descriptor is a long contiguous run per partition (~17.8 MB/core
instead of 64 MB).

Per DMA group the DVE does one wide bf16 multiply (2x packed mode) and one
half-fold add (prod[:512] + prod[512:]), halving what the reductions must
touch; per-partition row reductions run at 1x on every engine here, so the
512-element sums are split between ScalarE (activation Copy + accumulator,
~90% of chunks) and the DVE (tensor_scalar + add-accumulator), with the
last group's reductions alternated across both engines to shrink the tail.
The per-row mask/bias (c for valid rows, -1e10 for pad rows) is added
before a ScalarE exp, done in 32-column slabs as chunks complete so the
softmax chain overlaps the stream.  Per-batch normalization runs on-device
with host-supplied chunk->batch membership matrices: per-chunk and
per-batch sums are tiny TensorE matmuls, the reciprocal is scattered back
to chunks by a third, and a TensorE transpose puts probabilities in
[chunk, row] layout for the output DMA.  No max-subtraction is needed:
energies are ~N(0,1) so exp is safe in f32.
"""

import numpy as np
import ml_dtypes

import concourse.bacc as bacc
import concourse.bass as bass
import concourse.tile as tile
from concourse import mybir
from concourse.bass_utils import run_bass_kernel_spmd

B, S, H = 64, 2048, 1024
NCORES = 8
CHUNK = 128
NEG = -1.0e10
GROUP = 8            # chunks per DMA transfer (2 MiB bf16)

f32 = mybir.dt.float32
f16 = mybir.dt.bfloat16          # 16-bit stream dtype (device)
np16 = ml_dtypes.bfloat16        # matching numpy dtype (host)

# chunk compute path: "cdve" = fused custom-DVE tensor_tensor_reduce;
# "split" = DVE multiply + reduction split between ScalarE accum / DVE reduce
PATH = "split"
SCALAR_FRAC = 0.55               # split path: fraction of chunks on ScalarE
PE_BLOCKS = 0                    # 512-row blocks (4 chunks each) on TensorE
BROWS = 4 * CHUNK                # rows per PE block

_cached = {}


def _plan(lens):
    """LPT-pack whole batches onto cores by chunk count."""
    chunks = [(L + CHUNK - 1) // CHUNK for L in lens]
    order = sorted(range(B), key=lambda i: -chunks[i])
    bins = [[] for _ in range(NCORES)]
    loads = [0] * NCORES
    for i in order:
        k = loads.index(min(loads))
        bins[k].append(i)
        loads[k] += chunks[i]
    ncol = max(loads)
    maxb = max(len(bn) for bn in bins)
    assert ncol <= 128 and maxb <= 128
    return chunks, bins, ncol, maxb


def _groups(ncol):
    """(start, size) DMA groups.

    Small groups first so compute starts right away (pipeline ramp), then
    full-size groups, and a small remainder last to shrink the tail."""
    sizes = []
    for s in (2, 2, 4):
        if sum(sizes) + s <= ncol:
            sizes.append(s)
    while ncol - sum(sizes) >= GROUP:
        sizes.append(GROUP)
    if ncol - sum(sizes):
        sizes.append(ncol - sum(sizes))
    out = []
    c = 0
    for s in sizes:
        out.append((c, s))
        c += s
    return out


def _build(ncol, maxb):
    nc = bacc.Bacc("TRN2", target_bir_lowering=False, debug=False,
                   num_devices=NCORES)

    x = nc.dram_tensor("x", [CHUNK, ncol, H], f16, kind="ExternalInput")
    v = nc.dram_tensor("v", [H], f16, kind="ExternalInput")
    addv = nc.dram_tensor("addv", [CHUNK, ncol], f32, kind="ExternalInput")
    mm = nc.dram_tensor("mm", [ncol, maxb], f32, kind="ExternalInput")
    mmt = nc.dram_tensor("mmt", [maxb, ncol], f32, kind="ExternalInput")
    ident = nc.dram_tensor("ident", [CHUNK, CHUNK], f32, kind="ExternalInput")
    out = nc.dram_tensor("out", [ncol, CHUNK], f32, kind="ExternalOutput")

    with tile.TileContext(nc) as tc:
        with tc.tile_pool(name="singles", bufs=1) as singles, \
             tc.tile_pool(name="xp", bufs=3) as xp, \
             tc.tile_pool(name="prodp", bufs=3) as prodp, \
             tc.tile_pool(name="junkp", bufs=2) as junkp, \
             tc.tile_pool(name="gjunkp", bufs=3) as gjunkp, \
             tc.tile_pool(name="dumpp", bufs=2) as dumpp, \
             tc.tile_pool(name="sp", bufs=2) as sp, \
             tc.tile_pool(name="pp", bufs=2, space="PSUM") as pp, \
             tc.tile_pool(name="ptp", bufs=1, space="PSUM") as ptp:

            # v replicated across all 128 partitions via 0-stride DMA
            vb = singles.tile([CHUNK, H], f16)
            v_ap = v.ap()
            v_bcast = bass.AP(tensor=v_ap.tensor, offset=v_ap.offset,
                              ap=[[0, CHUNK]] + list(v_ap.ap))
            nc.gpsimd.dma_start(out=vb, in_=v_bcast)

            # issue the first two x transfers before the small constant
            # loads so the multiply pipeline starts as early as possible
            groups = _groups(ncol)
            xt_pre = {}
            for gi in (0, 1):
                if gi < len(groups):
                    c0, gsz = groups[gi]
                    xt = xp.tile([CHUNK, gsz, H], f16)
                    eng = nc.sync if gi % 2 == 0 else nc.gpsimd
                    eng.dma_start(out=xt, in_=x[:, c0:c0 + gsz, :])
                    xt_pre[gi] = xt

            addvt = singles.tile([CHUNK, ncol], f32)
            nc.gpsimd.dma_start(out=addvt, in_=addv[:, :])
            mmtl = singles.tile([ncol, maxb], f32)
            nc.gpsimd.dma_start(out=mmtl, in_=mm[:, :])
            mmttl = singles.tile([maxb, ncol], f32)
            nc.gpsimd.dma_start(out=mmttl, in_=mmt[:, :])
            identt = singles.tile([CHUNK, CHUNK], f32)
            nc.gpsimd.dma_start(out=identt, in_=ident[:, :])
            ones = singles.tile([CHUNK, 1], f32)
            nc.vector.memset(ones, 1.0)

            # energies, one column per chunk
            e = singles.tile([CHUNK, ncol], f32)
            e2 = singles.tile([CHUNK, ncol], f32)
            p = singles.tile([CHUNK, ncol], f32)
            cs_ps = pp.tile([ncol, 1], f32)

            vb_ap = vb[:, :]

            done = 0             # columns already masked+exp'd+chunk-summed
            for gi, (c0, gsz) in enumerate(groups):
                if gi in xt_pre:
                    xt = xt_pre[gi]
                else:
                    xt = xp.tile([CHUNK, gsz, H], f16)
                    eng = nc.sync if gi % 2 == 0 else nc.gpsimd
                    eng.dma_start(out=xt, in_=x[:, c0:c0 + gsz, :])
                if PATH == "cdve":
                    from concourse.dve_ops import TENSOR_TENSOR_REDUCE
                    for n in range(gsz):
                        c = c0 + n
                        junk = junkp.tile([CHUNK, H], f16)
                        # e[:,c] = addv[:,c] + sum_h x[:,c,h] * v[h]
                        nc.vector._custom_dve(
                            TENSOR_TENSOR_REDUCE, out=junk,
                            in0=xt[:, n, :], in1=vb,
                            s0=addvt[:, c:c + 1], s1=1.0,
                            accum_out=e[:, c:c + 1])
                else:
                    # one wide multiply per group, then per-chunk reductions
                    vb_rep = bass.AP(tensor=vb_ap.tensor, offset=vb_ap.offset,
                                     ap=[vb_ap.ap[0], [0, gsz], vb_ap.ap[1]])
                    prod = prodp.tile([CHUNK, gsz, H], f16)
                    nc.vector.tensor_mul(prod, xt, vb_rep)
                    # fold halves once on DVE (2x) so every reduction
                    # touches 512 elements instead of 1024
                    nfold = gsz
                    fold = gjunkp.tile([CHUNK, nfold, H // 2], f16)
                    nc.vector.tensor_add(fold, prod[:, :nfold, :H // 2],
                                         prod[:, :nfold, H // 2:])
                    last = gi == len(groups) - 1
                    for n in range(gsz):
                        c = c0 + n
                        dve_lane = (n % 2 == 0) if last else ((c % 20) < 2)
                        if dve_lane and n < nfold:
                            # DVE lane: tensor_scalar copy body + add-accum
                            junk = junkp.tile([CHUNK, H // 2], f16)
                            nc.vector.tensor_scalar(
                                out=junk, in0=fold[:, n, :],
                                scalar1=1.0, scalar2=0.0,
                                op0=mybir.AluOpType.mult,
                                op1=mybir.AluOpType.add,
                                accum_out=e[:, c:c + 1])
                        else:
                            # ScalarE lane: activation copy + accumulator
                            src = fold[:, n, :] if n < nfold \
                                else prod[:, n, :]
                            wid = H // 2 if n < nfold else H
                            dump = dumpp.tile([CHUNK, wid], f16)
                            nc.scalar.activation(
                                out=dump, in_=src,
                                func=mybir.ActivationFunctionType.Copy,
                                accum_out=e[:, c:c + 1])
                    if c0 + gsz in (32, 64) and c0 + gsz < ncol:
                        # columns [k0,k) done: mask+exp+chunk-sum them now so
                        # most of the softmax chain overlaps the loop tail
                        # (PSUM out base partition must be 0/32/64)
                        k = c0 + gsz
                        k0 = done
                        done = k
                        nc.vector.tensor_add(e2[:, k0:k], e[:, k0:k],
                                             addvt[:, k0:k])
                        nc.scalar.activation(
                            out=p[:, k0:k], in_=e2[:, k0:k],
                            func=mybir.ActivationFunctionType.Exp)
                        nc.tensor.matmul(cs_ps[k0:k], p[:, k0:k], ones,
                                         start=True, stop=True)

            # p = exp(e + addv); pad rows/chunks get -1e10 -> p = 0 exactly.
            # Columns below `done` were exp'd + chunk-summed inside the loop.
            sp48 = done
            nc.vector.tensor_add(e2[:, sp48:], e[:, sp48:], addvt[:, sp48:])
            nc.scalar.activation(out=p[:, sp48:], in_=e2[:, sp48:],
                                 func=mybir.ActivationFunctionType.Exp)
            if sp48 < ncol:
                nc.tensor.matmul(cs_ps[sp48:ncol], p[:, sp48:], ones,
                                 start=True, stop=True)

            # transpose early: depends only on p, runs off the chain
            pt_ps = ptp.tile([ncol, CHUNK], f32)
            nc.tensor.transpose(pt_ps, p, identt)

            cs = sp.tile([ncol, 1], f32)
            nc.scalar.copy(cs, cs_ps)
            # per-batch sums: bs[b] = sum_c mm[c, b] * cs[c]
            bs_ps = pp.tile([maxb, 1], f32)
            nc.tensor.matmul(bs_ps, mmtl, cs, start=True, stop=True)
            bs = sp.tile([maxb, 1], f32)
            # unused batch slots sum to 0; clamp so 1/0 can't poison matmuls
            nc.vector.tensor_scalar_max(bs, bs_ps, 1.0e-30)
            rb = sp.tile([maxb, 1], f32)
            nc.vector.reciprocal(rb, bs)
            # scatter 1/sum back to chunks: sc[c] = sum_b mmt[b, c] * rb[b]
            sc_ps = pp.tile([ncol, 1], f32)
            nc.tensor.matmul(sc_ps, mmttl, rb, start=True, stop=True)
            sc = sp.tile([ncol, 1], f32)
            nc.scalar.copy(sc, sc_ps)

            # scale each chunk row of the transposed probabilities by sc
            outt = sp.tile([ncol, CHUNK], f32)
            nc.vector.tensor_scalar_mul(outt, pt_ps, sc)
            nc.sync.dma_start(out=out[:, :], in_=outt)

    nc.compile()
    return nc


def _get(text_lens):
    lens = tuple(int(t) for t in np.asarray(text_lens))
    if lens not in _cached:
        chunks, bins, ncol, maxb = _plan(lens)
        nc = _build(ncol, maxb)
        _cached[lens] = (nc, chunks, bins, ncol, maxb)
    return _cached[lens]


def _in_maps(nc, chunks, bins, ncol, maxb, outputs, lens, W, b, weight_vec):
    W = np.asarray(W)
    bb = np.asarray(b)
    wv = np.asarray(weight_vec)
    v = (W.astype(np.float64).T @ wv.astype(np.float64)).astype(np16)
    c = np.float32(wv.astype(np.float64) @ bb.astype(np.float64))
    x16 = np.asarray(outputs).astype(np16)
    ident = np.eye(CHUNK, dtype=np.float32)

    maps = []
    for k in range(NCORES):
        xlin = np.zeros((ncol * CHUNK, H), np16)
        alin = np.full(ncol * CHUNK, NEG, np.float32)
        m = np.zeros((ncol, maxb), np.float32)
        c0 = 0
        for j, bi in enumerate(bins[k]):
            L = lens[bi]
            xlin[c0 * CHUNK:c0 * CHUNK + L] = x16[bi, :L]
            alin[c0 * CHUNK:c0 * CHUNK + L] = c
            m[c0:c0 + chunks[bi], j] = 1.0
            c0 += chunks[bi]
        xk = np.ascontiguousarray(
            xlin.reshape(ncol, CHUNK, H).transpose(1, 0, 2))
        ak = np.ascontiguousarray(alin.reshape(ncol, CHUNK).T)
        maps.append({"x": xk, "v": v, "addv": ak, "mm": m,
                     "mmt": np.ascontiguousarray(m.T), "ident": ident})
    return maps


def _gather(res, chunks, bins, lens):
    full = np.zeros((B, S), np.float32)
    for k in range(NCORES):
        flat = np.asarray(res.results[k]["out"]).reshape(-1)
        c0 = 0
        for bi in bins[k]:
            L = lens[bi]
            full[bi, :L] = flat[c0 * CHUNK:c0 * CHUNK + L]
            c0 += chunks[bi]
    return full


def kernel(outputs, text_lens, W, b, weight_vec):
    nc, chunks, bins, ncol, maxb = _get(text_lens)
    lens = [int(t) for t in np.asarray(text_lens)]
    maps = _in_maps(nc, chunks, bins, ncol, maxb, outputs, lens, W, b,
                    weight_vec)
    res = run_bass_kernel_spmd(nc, maps, list(range(NCORES)))
    return _gather(res, chunks, bins, lens)


def kernel_traced(outputs, text_lens, W, b, weight_vec, **trace_kwargs):
    """Like kernel() but profiles the run; returns (output, results)."""
    nc, chunks, bins, ncol, maxb = _get(text_lens)
    lens = [int(t) for t in np.asarray(text_lens)]
    maps = _in_maps(nc, chunks, bins, ncol, maxb, outputs, lens, W, b,
                    weight_vec)
    res = run_bass_kernel_spmd(nc, maps, list(range(NCORES)), trace=True,
                               **trace_kwargs)
    return _gather(res, chunks, bins, lens), res



# revision 27
# speedup vs baseline: 1.9620x; 1.9620x over previous
"""Trainium2 Bass kernel for nn_Attn_48206712930921.

softmax over s of energies[b,s] where energies[b,s] = outputs[b,s,:].v + c,
v = W^T @ weight_vec, c = weight_vec.b  (the [H,H] projection collapses to a
length-H dot product).  Rows s >= text_lens[b] softmax to exactly 0, so only
the valid prefix of each sequence is read (~49.5% of the input).

Layout/engine plan (v2, fp8 + TensorE):
  * Host packs each core's valid rows as fp8 e3m4 (relmax vs f32 reference
    ~2e-3, tolerance is 2e-2) in transposed [h, row] layout: DRAM
    x8[128, NBLK, 8*512] where partition k of block j holds slice s values
    x[row 512j+r, h=128s+k] as 4KB contiguous runs -> near-peak DMA.
  * v is scaled by 16 (keeps fp8 out of subnormals) and replicated across
    128 columns on the host; stationary lhsT = vrep[:, s, :] makes every
    PSUM partition receive the same energy row, so block j's energies are
    drained from partition j into a compact es[NBLK, 512] without any
    cross-partition moves.  8 accumulating matmuls per 512-row block
    (K=128 each), N=512, fp8 at bf16 speed ~216ns/MM.
  * Softmax: one DVE add of the pad mask (-1.6e11 on pad rows, 0 else),
    ScalarE Exp with scale=1/16 and bias=c, then chunk->batch membership
    matmuls in float32r: bs[b,f'] = sum_j memb_t[j,b] p[j,128t+f']
    accumulated over t, DVE-reduced, clamped, reciprocal, scattered back
    to per-chunk scales via the transposed membership matmuls, 4 DVE
    multiplies, and a single 34KB output DMA per core.
"""

import numpy as np
import ml_dtypes

import concourse.bacc as bacc
import concourse.bass as bass
import concourse.tile as tile
from concourse import mybir
from concourse.bass_utils import run_bass_kernel_spmd

B, S, H = 64, 2048, 1024
NCORES = 8
CHUNK = 128
BLK = 512                 # rows per block (= PSUM bank free size in f32)
CPB = BLK // CHUNK        # chunks per block = 4
NSL = H // CHUNK          # h slices = 8
VSCALE = 16.0
NEG16 = -1.6e11           # pad-row mask, pre-scaled by VSCALE
GRP = 2                   # blocks per x DMA (1 MiB)
WARMUP_MM = 16            # dummy matmuls to lift the PE HAM gate during ramp

f32 = mybir.dt.float32
f32r = mybir.dt.float32r
f8 = mybir.dt.float8e3
f16 = mybir.dt.bfloat16
np8 = ml_dtypes.float8_e3m4
npb16 = ml_dtypes.bfloat16

_cached = {}


def _plan(lens):
    """LPT-pack whole batches onto cores by chunk count."""
    chunks = [(L + CHUNK - 1) // CHUNK for L in lens]
    order = sorted(range(B), key=lambda i: -chunks[i])
    bins = [[] for _ in range(NCORES)]
    loads = [0] * NCORES
    for i in order:
        k = loads.index(min(loads))
        bins[k].append(i)
        loads[k] += chunks[i]
    ncol = max(loads)
    nblk = (ncol + CPB - 1) // CPB
    maxb = max(len(bn) for bn in bins)
    assert nblk <= 32 and maxb <= 128
    return chunks, bins, nblk, maxb


def _build(nblk, maxb):
    nc = bacc.Bacc("TRN2", target_bir_lowering=False, debug=False,
                   num_devices=NCORES)

    FREE = NSL * BLK  # 4096 fp8 bytes per partition per block
    x = nc.dram_tensor("x", [CHUNK, nblk, FREE], f8, kind="ExternalInput")
    v8 = nc.dram_tensor("v8", [CHUNK, NSL, 32], f8, kind="ExternalInput")
    addv = nc.dram_tensor("addv", [nblk, BLK], f32, kind="ExternalInput")
    memb = nc.dram_tensor("memb", [nblk, CPB, maxb], f16,
                          kind="ExternalInput")
    membt = nc.dram_tensor("membt", [maxb, CPB, nblk], f16,
                           kind="ExternalInput")
    cbias = nc.dram_tensor("cbias", [nblk, 1], f32, kind="ExternalInput")
    out = nc.dram_tensor("out", [nblk, BLK], f32, kind="ExternalOutput")

    # DMA groups: two single blocks for pipeline ramp, then 2-block (1 MiB)
    # aligned to PSUM-bank halves so matmul waves never straddle transfers
    groups = []
    j = 0
    for sz in (1, 1):
        if j < nblk:
            groups.append((j, min(sz, nblk - j)))
            j += sz
    while j < nblk:
        sz = min(2, nblk - j)
        groups.append((j, sz))
        j += sz
    blk2grp = {}
    for gi, (g0, gsz) in enumerate(groups):
        for jj in range(gsz):
            blk2grp[g0 + jj] = (gi, jj)
    nbank = (nblk + CPB - 1) // CPB  # PSUM bank groups of 4 blocks

    with tile.TileContext(nc) as tc:
        with tc.tile_pool(name="singles", bufs=1) as singles, \
             tc.tile_pool(name="xp", bufs=len(groups)) as xp, \
             tc.tile_pool(name="sp", bufs=2) as sp, \
             tc.tile_pool(name="pp", bufs=2, space="PSUM") as pp, \
             tc.tile_pool(name="pps", bufs=2, space="PSUM") as pps:

            # stationary v first on the x queue so matmuls can start early
            vt = singles.tile([CHUNK, NSL, 32], f8)
            nc.sync.dma_start(out=vt, in_=v8[:, :, :])

            # x stream
            xts = []
            for (g0, gsz) in groups:
                xt = xp.tile([CHUNK, gsz, FREE], f8)
                nc.sync.dma_start(out=xt, in_=x[:, g0:g0 + gsz, :])
                xts.append(xt)

            # small constants on the SWDGE queue in parallel
            addvt = singles.tile([nblk, BLK], f32)
            nc.gpsimd.dma_start(out=addvt, in_=addv[:, :])
            membl = singles.tile([nblk, CPB, maxb], f16)
            nc.gpsimd.dma_start(out=membl, in_=memb[:, :, :])
            membtl = singles.tile([maxb, CPB, nblk], f16)
            nc.gpsimd.dma_start(out=membtl, in_=membt[:, :, :])
            cbiast = singles.tile([nblk, 1], f32)
            nc.gpsimd.dma_start(out=cbiast, in_=cbias[:, :])
            onesb = singles.tile([maxb, CHUNK], f16)
            nc.vector.memset(onesb, 1.0)

            es = singles.tile([nblk, BLK], f32)
            dr = singles.tile([CHUNK, nbank, BLK], f32)

            # PE warmup: dummy matmuls while the first x DMA lands
            wsrc = singles.tile([CHUNK, BLK], f8)
            nc.vector.memset(wsrc, 0.0)
            wtile = pp.tile([CHUNK, BLK], f32)
            for w in range(WARMUP_MM):
                nc.tensor.matmul(wtile[0:32, :], vt[:, 0, :], wsrc,
                                 start=(w == 0), stop=(w == WARMUP_MM - 1))

            # main stream: per 4-block PSUM bank group, 8 accumulating
            # K=128 matmuls per block with M=1 and tile_position=(0,32m)
            # so the 4 blocks' columns run concurrently; the bank's first
            # matmul is the only start=True (bank-wide has_written clear),
            # every element's first touch then overwrites, later ones
            # accumulate.  ScalarE drains the full bank (base partition 0)
            # and small SBUF->SBUF DMAs gather rows {0,32,64,96} into the
            # compact es[nblk, 512].
            es_pitch = BLK
            dr_pitch = nbank * BLK
            for q in range(nbank):
                blocks = list(range(q * CPB, min((q + 1) * CPB, nblk)))
                bank = pp.tile([CHUNK, BLK], f32)
                for h in range(0, len(blocks), 2):
                    half = blocks[h:h + 2]
                    for s in range(NSL):
                        for hm, jb in enumerate(half):
                            m = h + hm
                            gi, jj = blk2grp[jb]
                            nc.tensor.matmul(
                                bank[32 * m:32 * m + 32, :], vt[:, s, :],
                                xts[gi][:, jj, s * BLK:(s + 1) * BLK],
                                start=(s == 0), stop=(s == NSL - 1),
                                tile_position=(0, 32 * m),
                                skip_group_check=True)
                    # drain + gather this half right away (off the tail)
                    p0 = 32 * h
                    nh = 32 * len(half)
                    nc.scalar.copy(dr[p0:p0 + nh, q, :], bank[p0:p0 + nh, :])
                    src = dr[p0:p0 + 1, q, :]
                    in_ap = bass.AP(tensor=src.tensor, offset=src.offset,
                                    ap=[[32 * dr_pitch, len(half)]]
                                    + list(src.ap)[1:])
                    e0 = q * CPB + h
                    nc.scalar.dma_start(out=es[e0:e0 + len(half), :],
                                        in_=in_ap)

            # p = exp((es + mask)/16 + c); pad rows -> exp(-1e10) = 0
            e2 = singles.tile([nblk, BLK], f32)
            nc.vector.tensor_add(e2, es, addvt)
            p16 = singles.tile([nblk, BLK], f16)
            nc.scalar.activation(out=p16, in_=e2,
                                 func=mybir.ActivationFunctionType.Exp,
                                 scale=1.0 / VSCALE, bias=cbiast)

            # batch sums: bs[b, f'] = sum_{j,t in batch} p[j, 128t+f']
            bs_ps = pps.tile([maxb, CHUNK], f32)
            for t in range(CPB):
                nc.tensor.matmul(bs_ps, membl[:, t, :],
                                 p16[:, t * CHUNK:(t + 1) * CHUNK],
                                 start=(t == 0), stop=(t == CPB - 1))
            junk = sp.tile([maxb, CHUNK], f32)
            bs = sp.tile([maxb, 1], f32)
            nc.vector.tensor_scalar(out=junk, in0=bs_ps, scalar1=1.0,
                                    scalar2=0.0, op0=mybir.AluOpType.mult,
                                    op1=mybir.AluOpType.add, accum_out=bs)
            bs2 = sp.tile([maxb, 1], f32)
            nc.vector.tensor_scalar_max(bs2, bs, 1.0e-30)
            rb = sp.tile([maxb, 1], f32)
            nc.vector.reciprocal(rb, bs2)
            # broadcast 1/bs along 128 columns, scatter to chunk scales
            rbb = sp.tile([maxb, CHUNK], f16)
            nc.vector.tensor_scalar_mul(rbb, onesb, rb)
            pn = singles.tile([nblk, BLK], f32)
            for t in range(CPB):
                ss_ps = pps.tile([nblk, CHUNK], f32)
                nc.tensor.matmul(ss_ps, membtl[:, t, :], rbb,
                                 start=True, stop=True)
                nc.vector.tensor_mul(pn[:, t * CHUNK:(t + 1) * CHUNK],
                                     p16[:, t * CHUNK:(t + 1) * CHUNK], ss_ps)
                nc.sync.dma_start(out=out[:, t * CHUNK:(t + 1) * CHUNK],
                                  in_=pn[:, t * CHUNK:(t + 1) * CHUNK])

    nc.compile()
    return nc


def _get(text_lens):
    lens = tuple(int(t) for t in np.asarray(text_lens))
    if lens not in _cached:
        chunks, bins, nblk, maxb = _plan(lens)
        nc = _build(nblk, maxb)
        _cached[lens] = (nc, chunks, bins, nblk, maxb)
    return _cached[lens]


def _in_maps(nc, chunks, bins, nblk, maxb, outputs, lens, W, b, weight_vec):
    W = np.asarray(W)
    bb = np.asarray(b)
    wv = np.asarray(weight_vec)
    v = (W.astype(np.float64).T @ wv.astype(np.float64))
    c = np.float32(wv.astype(np.float64) @ bb.astype(np.float64))
    v8 = np.clip(v * VSCALE, -28.0, 28.0).astype(np8)
    v8t = np.zeros((CHUNK, NSL, 32), np8)   # v in col 0, zeros elsewhere
    v8t[:, :, 0] = v8.reshape(NSL, CHUNK).T
    x_f32 = np.asarray(outputs)
    cb = np.full((nblk, 1), c, np.float32)

    R = nblk * BLK
    maps = []
    for k in range(NCORES):
        xlin = np.zeros((R, H), np8)
        alin = np.full(R, NEG16, np.float32)
        m = np.zeros((nblk * CPB, maxb), np.float32)
        c0 = 0
        for j, bi in enumerate(bins[k]):
            L = lens[bi]
            xlin[c0 * CHUNK:c0 * CHUNK + L] = np.clip(
                x_f32[bi, :L], -28.0, 28.0).astype(np8)
            alin[c0 * CHUNK:c0 * CHUNK + L] = 0.0
            m[c0:c0 + chunks[bi], j] = 1.0
            c0 += chunks[bi]
        # x8[k, j, s*512+r] = x[row 512j+r, h=128s+k]
        xk = np.ascontiguousarray(
            xlin.reshape(nblk, BLK, NSL, CHUNK).transpose(3, 0, 2, 1)
            .reshape(CHUNK, nblk, NSL * BLK))
        ak = np.ascontiguousarray(alin.reshape(nblk, BLK))
        mm = np.ascontiguousarray(m.reshape(nblk, CPB, maxb).astype(npb16))
        mmt = np.ascontiguousarray(mm.transpose(2, 1, 0))     # [maxb, 4, nblk]
        maps.append({"x": xk, "v8": v8t, "addv": ak, "memb": mm,
                     "membt": mmt, "cbias": cb})
    return maps


def _gather(res, chunks, bins, lens):
    full = np.zeros((B, S), np.float32)
    for k in range(NCORES):
        flat = np.asarray(res.results[k]["out"]).reshape(-1)
        c0 = 0
        for bi in bins[k]:
            L = lens[bi]
            full[bi, :L] = flat[c0 * CHUNK:c0 * CHUNK + L]
            c0 += chunks[bi]
    return full


def kernel(outputs, text_lens, W, b, weight_vec):
    nc, chunks, bins, nblk, maxb = _get(text_lens)
    lens = [int(t) for t in np.asarray(text_lens)]
    maps = _in_maps(nc, chunks, bins, nblk, maxb, outputs, lens, W, b,
                    weight_vec)
    res = run_bass_kernel_spmd(nc, maps, list(range(NCORES)))
    return _gather(res, chunks, bins, lens)


def kernel_traced(outputs, text_lens, W, b, weight_vec, **trace_kwargs):
    """Like kernel() but profiles the run; returns (output, results)."""
    nc, chunks, bins, nblk, maxb = _get(text_lens)
    lens = [int(t) for t in np.asarray(text_lens)]
    maps = _in_maps(nc, chunks, bins, nblk, maxb, outputs, lens, W, b,
                    weight_vec)
    res = run_bass_kernel_spmd(nc, maps, list(range(NCORES)), trace=True,
                               **trace_kwargs)
    return _gather(res, chunks, bins, lens), res


# revision 30
# speedup vs baseline: 2.0157x; 1.0274x over previous
"""Trainium2 Bass kernel for nn_Attn_48206712930921.

softmax over s of energies[b,s] where energies[b,s] = outputs[b,s,:].v + c,
v = W^T @ weight_vec, c = weight_vec.b  (the [H,H] projection collapses to a
length-H dot product).  Rows s >= text_lens[b] softmax to exactly 0, so only
the valid prefix of each sequence is read (~49.5% of the input).

Layout/engine plan (v2, fp8 + TensorE):
  * Host packs each core's valid rows as fp8 e3m4 (relmax vs f32 reference
    ~2e-3, tolerance is 2e-2) in transposed [h, row] layout: DRAM
    x8[128, NBLK, 8*512] where partition k of block j holds slice s values
    x[row 512j+r, h=128s+k] as 4KB contiguous runs -> near-peak DMA.
  * v is scaled by 16 (keeps fp8 out of subnormals) and replicated across
    128 columns on the host; stationary lhsT = vrep[:, s, :] makes every
    PSUM partition receive the same energy row, so block j's energies are
    drained from partition j into a compact es[NBLK, 512] without any
    cross-partition moves.  8 accumulating matmuls per 512-row block
    (K=128 each), N=512, fp8 at bf16 speed ~216ns/MM.
  * Softmax: one DVE add of the pad mask (-1.6e11 on pad rows, 0 else),
    ScalarE Exp with scale=1/16 and bias=c, then chunk->batch membership
    matmuls in float32r: bs[b,f'] = sum_j memb_t[j,b] p[j,128t+f']
    accumulated over t, DVE-reduced, clamped, reciprocal, scattered back
    to per-chunk scales via the transposed membership matmuls, 4 DVE
    multiplies, and a single 34KB output DMA per core.
"""

import numpy as np
import ml_dtypes

import concourse.bacc as bacc
import concourse.bass as bass
import concourse.tile as tile
from concourse import mybir
from concourse.bass_utils import run_bass_kernel_spmd

B, S, H = 64, 2048, 1024
NCORES = 8
CHUNK = 128
BLK = 512                 # rows per block (= PSUM bank free size in f32)
CPB = BLK // CHUNK        # chunks per block = 4
NSL = H // CHUNK          # h slices = 8
VSCALE = 16.0
NEG16 = -1.6e11           # pad-row mask, pre-scaled by VSCALE
GRP = 2                   # blocks per x DMA (1 MiB)
WARMUP_MM = 16            # dummy matmuls to lift the PE HAM gate during ramp

f32 = mybir.dt.float32
f32r = mybir.dt.float32r
f8 = mybir.dt.float8e3
f16 = mybir.dt.bfloat16
np8 = ml_dtypes.float8_e3m4
npb16 = ml_dtypes.bfloat16

_cached = {}


def _plan(lens):
    """LPT-pack whole batches onto cores by chunk count."""
    chunks = [(L + CHUNK - 1) // CHUNK for L in lens]
    order = sorted(range(B), key=lambda i: -chunks[i])
    bins = [[] for _ in range(NCORES)]
    loads = [0] * NCORES
    for i in order:
        k = loads.index(min(loads))
        bins[k].append(i)
        loads[k] += chunks[i]
    ncol = max(loads)
    nblk = (ncol + CPB - 1) // CPB
    maxb = max(len(bn) for bn in bins)
    assert nblk <= 32 and maxb <= 128
    return chunks, bins, nblk, maxb


def _build(nblk, maxb):
    nc = bacc.Bacc("TRN2", target_bir_lowering=False, debug=False,
                   num_devices=NCORES)

    FREE = NSL * BLK  # 4096 fp8 bytes per partition per block
    x = nc.dram_tensor("x", [CHUNK, nblk, FREE], f8, kind="ExternalInput")
    v8 = nc.dram_tensor("v8", [CHUNK, NSL, 32], f8, kind="ExternalInput")
    addv = nc.dram_tensor("addv", [nblk, BLK], f32, kind="ExternalInput")
    memb = nc.dram_tensor("memb", [nblk, CPB, maxb], f16,
                          kind="ExternalInput")
    membt = nc.dram_tensor("membt", [maxb, CPB, nblk], f16,
                           kind="ExternalInput")
    cbias = nc.dram_tensor("cbias", [nblk, 1], f32, kind="ExternalInput")
    out = nc.dram_tensor("out", [nblk, BLK], f32, kind="ExternalOutput")

    # DMA groups: two single blocks for pipeline ramp, then 2-block (1 MiB)
    # aligned to PSUM-bank halves so matmul waves never straddle transfers
    groups = []
    j = 0
    for sz in (1, 1):
        if j < nblk:
            groups.append((j, min(sz, nblk - j)))
            j += sz
    while j < nblk:
        sz = min(2, nblk - j)
        groups.append((j, sz))
        j += sz
    blk2grp = {}
    for gi, (g0, gsz) in enumerate(groups):
        for jj in range(gsz):
            blk2grp[g0 + jj] = (gi, jj)
    nbank = (nblk + CPB - 1) // CPB  # PSUM bank groups of 4 blocks

    with tile.TileContext(nc) as tc:
        with tc.tile_pool(name="singles", bufs=1) as singles, \
             tc.tile_pool(name="xp", bufs=len(groups)) as xp, \
             tc.tile_pool(name="sp", bufs=2) as sp, \
             tc.tile_pool(name="pp", bufs=2, space="PSUM") as pp, \
             tc.tile_pool(name="pps", bufs=2, space="PSUM") as pps:

            # stationary v first on the x queue so matmuls can start early
            vt = singles.tile([CHUNK, NSL, 32], f8)
            nc.sync.dma_start(out=vt, in_=v8[:, :, :])

            # x stream
            xts = []
            for (g0, gsz) in groups:
                xt = xp.tile([CHUNK, gsz, FREE], f8)
                nc.sync.dma_start(out=xt, in_=x[:, g0:g0 + gsz, :])
                xts.append(xt)

            # small constants on the SWDGE queue in parallel
            addvt = singles.tile([nblk, BLK], f32)
            nc.gpsimd.dma_start(out=addvt, in_=addv[:, :])
            membl = singles.tile([nblk, CPB, maxb], f16)
            nc.gpsimd.dma_start(out=membl, in_=memb[:, :, :])
            membtl = singles.tile([maxb, CPB, nblk], f16)
            nc.gpsimd.dma_start(out=membtl, in_=membt[:, :, :])
            cbiast = singles.tile([nblk, 1], f32)
            nc.gpsimd.dma_start(out=cbiast, in_=cbias[:, :])
            onesb = singles.tile([maxb, CHUNK], f16)
            nc.vector.memset(onesb, 1.0)

            es = singles.tile([nblk, BLK], f32)
            dr = singles.tile([CHUNK, nbank, BLK], f32)

            # PE warmup: dummy matmuls while the first x DMA lands
            wsrc = singles.tile([CHUNK, BLK], f8)
            nc.vector.memset(wsrc, 0.0)
            wtile = pp.tile([CHUNK, BLK], f32)
            for w in range(WARMUP_MM):
                nc.tensor.matmul(wtile[0:32, 0:CHUNK], vt[:, 0, :],
                                 wsrc[:, 0:CHUNK],
                                 start=(w == 0), stop=(w == WARMUP_MM - 1))

            # main stream: per 4-block PSUM bank group, 8 accumulating
            # K=128 matmuls per block with M=1 and tile_position=(0,32m)
            # so the 4 blocks' columns run concurrently; the bank's first
            # matmul is the only start=True (bank-wide has_written clear),
            # every element's first touch then overwrites, later ones
            # accumulate.  ScalarE drains the full bank (base partition 0)
            # and small SBUF->SBUF DMAs gather rows {0,32,64,96} into the
            # compact es[nblk, 512].
            es_pitch = BLK
            dr_pitch = nbank * BLK
            for q in range(nbank):
                blocks = list(range(q * CPB, min((q + 1) * CPB, nblk)))
                bank = pp.tile([CHUNK, BLK], f32)
                for h in range(0, len(blocks), 2):
                    half = blocks[h:h + 2]
                    for s in range(NSL):
                        for hm, jb in enumerate(half):
                            m = h + hm
                            gi, jj = blk2grp[jb]
                            nc.tensor.matmul(
                                bank[32 * m:32 * m + 32, :], vt[:, s, :],
                                xts[gi][:, jj, s * BLK:(s + 1) * BLK],
                                start=(s == 0), stop=(s == NSL - 1),
                                tile_position=(0, 32 * m),
                                skip_group_check=True)
                    # drain + gather this half right away (off the tail)
                    p0 = 32 * h
                    nh = 32 * len(half)
                    nc.scalar.copy(dr[p0:p0 + nh, q, :], bank[p0:p0 + nh, :])
                    src = dr[p0:p0 + 1, q, :]
                    in_ap = bass.AP(tensor=src.tensor, offset=src.offset,
                                    ap=[[32 * dr_pitch, len(half)]]
                                    + list(src.ap)[1:])
                    e0 = q * CPB + h
                    nc.scalar.dma_start(out=es[e0:e0 + len(half), :],
                                        in_=in_ap)

            # p = exp((es + mask)/16 + c); pad rows -> exp(-1e10) = 0
            e2 = singles.tile([nblk, BLK], f32)
            nc.vector.tensor_add(e2, es, addvt)
            p16 = singles.tile([nblk, BLK], f16)
            nc.scalar.activation(out=p16, in_=e2,
                                 func=mybir.ActivationFunctionType.Exp,
                                 scale=1.0 / VSCALE, bias=cbiast)

            # batch sums: bs[b, f'] = sum_{j,t in batch} p[j, 128t+f']
            bs_ps = pps.tile([maxb, CHUNK], f32)
            for t in range(CPB):
                nc.tensor.matmul(bs_ps, membl[:, t, :],
                                 p16[:, t * CHUNK:(t + 1) * CHUNK],
                                 start=(t == 0), stop=(t == CPB - 1))
            junk = sp.tile([maxb, CHUNK], f32)
            bs = sp.tile([maxb, 1], f32)
            nc.vector.tensor_scalar(out=junk, in0=bs_ps, scalar1=1.0,
                                    scalar2=0.0, op0=mybir.AluOpType.mult,
                                    op1=mybir.AluOpType.add, accum_out=bs)
            bs2 = sp.tile([maxb, 1], f32)
            nc.vector.tensor_scalar_max(bs2, bs, 1.0e-30)
            rb = sp.tile([maxb, 1], f32)
            nc.vector.reciprocal(rb, bs2)
            # broadcast 1/bs along 128 columns, scatter to chunk scales
            rbb = sp.tile([maxb, CHUNK], f16)
            nc.vector.tensor_scalar_mul(rbb, onesb, rb)
            pn = singles.tile([nblk, BLK], f32)
            for t in range(CPB):
                ss_ps = pps.tile([nblk, CHUNK], f32)
                nc.tensor.matmul(ss_ps, membtl[:, t, :], rbb,
                                 start=True, stop=True)
                nc.vector.tensor_mul(pn[:, t * CHUNK:(t + 1) * CHUNK],
                                     p16[:, t * CHUNK:(t + 1) * CHUNK], ss_ps)
                nc.sync.dma_start(out=out[:, t * CHUNK:(t + 1) * CHUNK],
                                  in_=pn[:, t * CHUNK:(t + 1) * CHUNK])

    nc.compile()
    return nc


def _get(text_lens):
    lens = tuple(int(t) for t in np.asarray(text_lens))
    if lens not in _cached:
        chunks, bins, nblk, maxb = _plan(lens)
        nc = _build(nblk, maxb)
        _cached[lens] = (nc, chunks, bins, nblk, maxb)
    return _cached[lens]


def _in_maps(nc, chunks, bins, nblk, maxb, outputs, lens, W, b, weight_vec):
    W = np.asarray(W)
    bb = np.asarray(b)
    wv = np.asarray(weight_vec)
    v = (W.astype(np.float64).T @ wv.astype(np.float64))
    c = np.float32(wv.astype(np.float64) @ bb.astype(np.float64))
    v8 = np.clip(v * VSCALE, -28.0, 28.0).astype(np8)
    v8t = np.zeros((CHUNK, NSL, 32), np8)   # v in col 0, zeros elsewhere
    v8t[:, :, 0] = v8.reshape(NSL, CHUNK).T
    x_f32 = np.asarray(outputs)
    cb = np.full((nblk, 1), c, np.float32)

    R = nblk * BLK
    maps = []
    for k in range(NCORES):
        xlin = np.zeros((R, H), np8)
        alin = np.full(R, NEG16, np.float32)
        m = np.zeros((nblk * CPB, maxb), np.float32)
        c0 = 0
        for j, bi in enumerate(bins[k]):
            L = lens[bi]
            xlin[c0 * CHUNK:c0 * CHUNK + L] = np.clip(
                x_f32[bi, :L], -28.0, 28.0).astype(np8)
            alin[c0 * CHUNK:c0 * CHUNK + L] = 0.0
            m[c0:c0 + chunks[bi], j] = 1.0
            c0 += chunks[bi]
        # x8[k, j, s*512+r] = x[row 512j+r, h=128s+k]
        xk = np.ascontiguousarray(
            xlin.reshape(nblk, BLK, NSL, CHUNK).transpose(3, 0, 2, 1)
            .reshape(CHUNK, nblk, NSL * BLK))
        ak = np.ascontiguousarray(alin.reshape(nblk, BLK))
        mm = np.ascontiguousarray(m.reshape(nblk, CPB, maxb).astype(npb16))
        mmt = np.ascontiguousarray(mm.transpose(2, 1, 0))     # [maxb, 4, nblk]
        maps.append({"x": xk, "v8": v8t, "addv": ak, "memb": mm,
                     "membt": mmt, "cbias": cb})
    return maps


def _gather(res, chunks, bins, lens):
    full = np.zeros((B, S), np.float32)
    for k in range(NCORES):
        flat = np.asarray(res.results[k]["out"]).reshape(-1)
        c0 = 0
        for bi in bins[k]:
            L = lens[bi]
            full[bi, :L] = flat[c0 * CHUNK:c0 * CHUNK + L]
            c0 += chunks[bi]
    return full


def kernel(outputs, text_lens, W, b, weight_vec):
    nc, chunks, bins, nblk, maxb = _get(text_lens)
    lens = [int(t) for t in np.asarray(text_lens)]
    maps = _in_maps(nc, chunks, bins, nblk, maxb, outputs, lens, W, b,
                    weight_vec)
    res = run_bass_kernel_spmd(nc, maps, list(range(NCORES)))
    return _gather(res, chunks, bins, lens)


def kernel_traced(outputs, text_lens, W, b, weight_vec, **trace_kwargs):
    """Like kernel() but profiles the run; returns (output, results)."""
    nc, chunks, bins, nblk, maxb = _get(text_lens)
    lens = [int(t) for t in np.asarray(text_lens)]
    maps = _in_maps(nc, chunks, bins, nblk, maxb, outputs, lens, W, b,
                    weight_vec)
    res = run_bass_kernel_spmd(nc, maps, list(range(NCORES)), trace=True,
                               **trace_kwargs)
    return _gather(res, chunks, bins, lens), res
